# revision 2
# baseline (speedup 1.0000x reference)
"""Trainium2 Bass kernel for nn_CentersDistance (retrieval_knn).

logits[k, n] = -||centers[k] - inputs[n]||^2
             = 2*(centers @ inputs.T)[k, n] - ||centers[k]||^2 - ||inputs[n]||^2

Strategy (8 NeuronCores, data-parallel over the N=8192 inputs): fp8
DoubleRow GEMM (64 matmuls of [128p x 256c x 512f], PSUM fp32), exact
f64 norm terms added in a PSUM->SBUF epilogue, bf16/fp16 output.

v2 changes vs the 31.7us baseline (trace-driven):
  * loads are 128KB chunks with per-chunk semaphores, ordered so the
    d0 tile pair (xta[0] on the Sync HW queue, cta[0] on the Scalar HW
    queue) is the FIRST thing each queue moves.  Measured: per-queue
    streaming is only ~95-130 B/ns and first packets appear ~1.4us
    after dispatch, so the baseline's 256KB-tile + ncsq-first order
    made the first real matmul wait until 12.1us; chunking pulls it
    to ~10.0us.
  * ct is split m0-3 (cta, pass 1) / m4-7 (ctb, pass 2): pass 1 only
    needs half of each ct d-tile, pass 2's half arrives later with
    slack (Scalar: ctb0,1; ring: ctb2,3).
  * pass 1 is d-outer / h-outer / m-inner (first 4 matmuls need only
    xta[0]+cta[0] = 256KB); pass 2 is m-outer / d-inner so each
    m-tile's two groups retire every ~1.7us for store overlap, with
    PSUM bank handoff waits matched to the drain schedule.
  * the LAST m-tile (m7) runs h0 (bank6), then h1 as two 256-col
    sub-groups in two different freed banks (7 and 0), so the final
    drain after the last matmul is a [128,256] op, and the final
    stores are 64KB quarters spread across all three DMA queues.
  * epilogue is spread over three engines: DVE direct STTs + finishes,
    Act PSUM drains (Identity + ncsq bias -> fp16 tmp), GpSimd
    tensor_add finishes (SBUF-only).  Assignments are chosen so no
    engine queue backs up at the tail and Act/DVE never touch the
    same PSUM bank concurrently (P10: concurrent access of one bank
    from two engines is unsafe; PE-write + engine-read is fatal).
  * PE warmup matmuls (N=256, ~213ns each cold) bridge the preamble
    until the first chunks land and keep the HAM clock ramp running.

Measured window note: exec_time = last semaphore teardown event minus
first post-preamble instruction; the ~6.8us all-sems reset storm the
NEFF epilogue emits is fixed-length, so kernel-body savings move the
reported number 1:1.

Accuracy: identical quantization to baseline (fp8e4m3 cross term,
exact f64 norms, fp16 store) -> absmax/scale ~5.2e-3 vs 2e-2 gate.
"""

import threading
from contextlib import ExitStack

import numpy as np
import ml_dtypes

import concourse.mybir as mybir
from concourse import bacc
from concourse.bass_utils import run_bass_kernel_spmd

N_CORES = 8
N, K, D = 8192, 1024, 1024
NSH = N // N_CORES  # per-core slab of inputs
P = 128             # SBUF partitions
NF = 512            # matmul moving free dim (one fp32 PSUM bank)
HNF = NF // 2

DP_TILES = D // (2 * P)  # 4 double-row contraction tiles (256 deep each)
M_TILES = K // P         # 8 center tiles

N_WU = 14           # warmup matmuls (N=256, ~213ns each cold)
N_TMP = 6           # fp16 staging buffers for Act->finish pipeline

_DT = mybir.dt.float8e4
_NP_DT = ml_dtypes.float8_e4m3
_OUT_DT = mybir.dt.float16
_DR = mybir.MatmulPerfMode.DoubleRow

_cache = threading.local()


def _build_nc():
    nc = bacc.Bacc(
        "TRN2", target_bir_lowering=False, debug=False, num_devices=N_CORES
    )
    cta = nc.dram_tensor("cta", [DP_TILES, P, 2, NF], _DT, kind="ExternalInput").ap()
    ctb = nc.dram_tensor("ctb", [DP_TILES, P, 2, NF], _DT, kind="ExternalInput").ap()
    xta = nc.dram_tensor("xta", [DP_TILES, P, 2, NF], _DT, kind="ExternalInput").ap()
    xtb = nc.dram_tensor("xtb", [DP_TILES, P, 2, NF], _DT, kind="ExternalInput").ap()
    ncsq = nc.dram_tensor(
        "ncsq", [P, M_TILES], mybir.dt.float32, kind="ExternalInput"
    ).ap()
    nxsq = nc.dram_tensor(
        "nxsq", [P, NSH], mybir.dt.float16, kind="ExternalInput"
    ).ap()
    out = nc.dram_tensor("out", [K, NSH], _OUT_DT, kind="ExternalOutput").ap()
    out_r = out.rearrange("(m p) n -> m p n", p=P)

    with (
        nc.sbuf_tensor("wu_sb", [P, 2, 256], _DT) as wu_sb,
        nc.sbuf_tensor("ncsq_sb", [P, M_TILES], mybir.dt.float32) as ncsq_sb,
        nc.sbuf_tensor("nxsq_sb", [P, NSH], mybir.dt.float16) as nxsq_sb,
        nc.sbuf_tensor("tmp_sb", [P, N_TMP, NF], mybir.dt.float16) as tmp_sb,
        # ot layout: [(m, h)] -> col block 2m+h, so each m-tile's store
        # is one contiguous [128, 1024] fp16 DMA (2KB lines).
        nc.sbuf_tensor("ot_sb", [P, 2 * M_TILES * NF], _OUT_DT) as ot_sb,
        ExitStack() as stack,
        nc.semaphore("const_sem") as const_sem,  # ncsq landed
        nc.semaphore("bc_sem") as bc_sem,        # nxsq landed
        nc.semaphore("mm_sem") as mm_sem,        # group stop-matmul retired
        nc.semaphore("ob_sem") as ob_sem,        # Act drain done (PSUM free)
        nc.semaphore("od_sem") as od_sem,        # DVE finalize count
        nc.semaphore("og_sem") as og_sem,        # GpSimd finalize count
        nc.semaphore("ctb_sem") as ctb_sem,      # all 4 chunks -> 64
        nc.semaphore("ds_sync") as ds_sync,
        nc.semaphore("ds_ring") as ds_ring,
        nc.semaphore("ds_scalar") as ds_scalar,
        nc.Block() as block,
    ):
        cta_sems = [
            stack.enter_context(nc.semaphore(f"cta_sem{d}")) for d in range(DP_TILES)
        ]
        xta_sems = [
            stack.enter_context(nc.semaphore(f"xta_sem{d}")) for d in range(DP_TILES)
        ]
        xtb_sems = [
            stack.enter_context(nc.semaphore(f"xtb_sem{d}")) for d in range(DP_TILES)
        ]
        ct_sb = [
            stack.enter_context(nc.sbuf_tensor(f"ct_sb{d}", [P, 2, K], _DT))
            for d in range(DP_TILES)
        ]
        xt_sb = [
            stack.enter_context(nc.sbuf_tensor(f"xt_sb{d}", [P, 2, NSH], _DT))
            for d in range(DP_TILES)
        ]
        ps = [
            stack.enter_context(nc.psum_tensor(f"ps{b}", [P, NF], mybir.dt.float32))
            for b in range(8)
        ]

        # ---- schedules -------------------------------------------------
        # mm_sem increment order (one inc per group's stop matmul):
        #  1 m0h0  2 m1h0  3 m2h0  4 m3h0  5 m0h1  6 m1h1  7 m2h1  8 m3h1
        #  9 m4h0 10 m4h1 11 m5h0 12 m5h1 13 m6h0 14 m6h1
        # 15 m7h0 16 m7h1a(bank7, cols 512:768) 17 m7h1b(bank0, 768:1024)
        #
        # Act drain order (ob):  m1h0, m2h0, m3h0, m0h1, m1h1, m2h1,
        #   m3h1, m4h1, m5h1, m6h1, m7h1a           (11 drains -> tmp)
        # DVE order (od): D m0h0, F m1h0, F m2h0, F m3h0, D m4h0,
        #   D m5h0, F m4h1, D m6h0, D m7h0, D m7h1b, F m7h1a
        # GpSimd finishes (og): m0h1, m1h1, m2h1, m3h1, m5h1, m6h1

        ACT_LIST = [  # (name, mm_wait, bank, psum_lo, psum_hi, ob_after)
            ("m1h0", 2, 1, 0, NF, 1),
            ("m2h0", 3, 2, 0, NF, 2),
            ("m3h0", 4, 3, 0, NF, 3),
            ("m0h1", 5, 4, 0, NF, 4),
            ("m1h1", 6, 5, 0, NF, 5),
            ("m2h1", 7, 6, 0, NF, 6),
            ("m3h1", 8, 7, 0, NF, 7),
            ("m4h1", 10, 1, 0, NF, 8),
            ("m5h1", 12, 3, 0, NF, 9),
            ("m6h1", 14, 5, 0, NF, 10),
            ("m7h1a", 16, 7, 0, HNF, 11),
        ]
        # ncsq column for each drained group
        ACT_M = {
            "m1h0": 1, "m2h0": 2, "m3h0": 3, "m0h1": 0, "m1h1": 1,
            "m2h1": 2, "m3h1": 3, "m4h1": 4, "m5h1": 5, "m6h1": 6,
            "m7h1a": 7,
        }
        # output column block (in units of HNF) and width for each group
        def _oc(name):
            m = int(name[1])
            if name == "m7h1a":
                return (2 * m + 1) * NF, HNF
            if name == "m7h1b":
                return (2 * m + 1) * NF + HNF, HNF
            h = int(name[3])
            return (2 * m + h) * NF, NF

        # tmp slot per Act drain + reuse wait: slot j%6; for j>=6 wait
        # until the finish consuming ACT_LIST[j-6]'s tmp has run.
        # Finishes: m1h0->od2, m2h0->od3, m3h0->od4, m0h1->og1,
        #   m1h1->og2, m2h1->og3, m3h1->og4, m4h1->od7, m5h1->og5,
        #   m6h1->og6, m7h1a->od11
        FIN = {
            "m1h0": ("od", 2), "m2h0": ("od", 3), "m3h0": ("od", 4),
            "m0h1": ("og", 1), "m1h1": ("og", 2), "m2h1": ("og", 3),
            "m3h1": ("og", 4), "m4h1": ("od", 7), "m5h1": ("og", 5),
            "m6h1": ("og", 6), "m7h1a": ("od", 11),
        }
        BUF = {it[0]: j % N_TMP for j, it in enumerate(ACT_LIST)}

        def _nxsq_slice(name):
            lo, w = _oc(name)
            # output block (2m+h)*NF+off maps to slab cols h*NF+off
            m = int(name[1])
            slab = lo - 2 * m * NF
            return nxsq_sb[:, slab : slab + w]

        # ---- engine programs -------------------------------------------

        @block.sync
        def _(sync):
            # critical chunk first: xta[0] feeds the first 4 matmuls
            for d in range(DP_TILES):
                sync.dma_start(
                    xt_sb[d][:, :, 0:NF], xta[d]
                ).then_inc(xta_sems[d], 16)
            sync.dma_start(ncsq_sb[:], ncsq).then_inc(const_sem, 16)
            # stores: m0, m2, m4, m6, then m7 quarters
            sync.wait_ge(od_sem, 1)
            sync.wait_ge(og_sem, 1)
            sync.dma_start(out_r[0][:], ot_sb[:, 0 : 2 * NF]).then_inc(ds_sync, 16)
            sync.wait_ge(od_sem, 3)
            sync.wait_ge(og_sem, 3)
            sync.dma_start(
                out_r[2][:], ot_sb[:, 4 * NF : 6 * NF]
            ).then_inc(ds_sync, 16)
            sync.wait_ge(od_sem, 7)
            sync.dma_start(
                out_r[4][:], ot_sb[:, 8 * NF : 10 * NF]
            ).then_inc(ds_sync, 16)
            sync.wait_ge(od_sem, 8)
            sync.wait_ge(og_sem, 6)
            sync.dma_start(
                out_r[6][:], ot_sb[:, 12 * NF : 14 * NF]
            ).then_inc(ds_sync, 16)
            # m7 h0 first half (cols 0:256)
            sync.wait_ge(od_sem, 9)
            sync.dma_start(
                out_r[7][:, 0:HNF], ot_sb[:, 14 * NF : 14 * NF + HNF]
            ).then_inc(ds_sync, 16)
            # m7 h1b (cols 768:1024)
            sync.wait_ge(od_sem, 10)
            sync.dma_start(
                out_r[7][:, NF + HNF : 2 * NF],
                ot_sb[:, 15 * NF + HNF : 16 * NF],
            ).then_inc(ds_sync, 16)
            sync.wait_ge(ds_sync, 6 * 16)

        @block.scalar
        def _(scalar):
            for d in range(DP_TILES):
                scalar.dma_start(
                    ct_sb[d][:, :, 0:NF], cta[d]
                ).then_inc(cta_sems[d], 16)
            for d in (0, 1):
                scalar.dma_start(
                    ct_sb[d][:, :, NF:K], ctb[d]
                ).then_inc(ctb_sem, 16)
            # Act drains: tmp = Identity(ps + ncsq_bias), fp32 PSUM -> fp16
            scalar.wait_ge(const_sem, 16)
            for j, (name, mmw, bank, lo, hi, _ob) in enumerate(ACT_LIST):
                if j >= N_TMP:
                    dep, cnt = FIN[ACT_LIST[j - N_TMP][0]]
                    scalar.wait_ge(od_sem if dep == "od" else og_sem, cnt)
                scalar.wait_ge(mm_sem, mmw)
                nc.scalar.add(
                    tmp_sb[:, BUF[name], 0 : hi - lo],
                    ps[bank][:, lo:hi],
                    ncsq_sb[:, ACT_M[name] : ACT_M[name] + 1],
                ).then_inc(ob_sem, 1)
            # store m7h1a (cols 512:768)
            scalar.wait_ge(od_sem, 11)
            scalar.dma_start(
                out_r[7][:, NF : NF + HNF],
                ot_sb[:, 15 * NF : 15 * NF + HNF],
            ).then_inc(ds_scalar, 16)
            scalar.wait_ge(ds_scalar, 16)

        @block.gpsimd
        def _(gpsimd):
            for d in range(DP_TILES):
                gpsimd.dma_start(
                    xt_sb[d][:, :, NF:NSH], xtb[d]
                ).then_inc(xtb_sems[d], 16)
            for d in (2, 3):
                gpsimd.dma_start(
                    ct_sb[d][:, :, NF:K], ctb[d]
                ).then_inc(ctb_sem, 16)
            gpsimd.dma_start(nxsq_sb[:], nxsq).then_inc(bc_sem, 16)

            def fin(name, ob_cnt):
                gpsimd.wait_ge(ob_sem, ob_cnt)
                lo, w = _oc(name)
                nc.gpsimd.tensor_add(
                    ot_sb[:, lo : lo + w],
                    tmp_sb[:, BUF[name], 0:w],
                    _nxsq_slice(name),
                ).then_inc(og_sem, 1)

            gpsimd.wait_ge(bc_sem, 16)
            fin("m0h1", 4)   # og1
            fin("m1h1", 5)   # og2
            gpsimd.wait_ge(od_sem, 2)
            gpsimd.dma_start(
                out_r[1][:], ot_sb[:, 2 * NF : 4 * NF]
            ).then_inc(ds_ring, 16)
            fin("m2h1", 6)   # og3
            fin("m3h1", 7)   # og4
            gpsimd.wait_ge(od_sem, 4)
            gpsimd.dma_start(
                out_r[3][:], ot_sb[:, 6 * NF : 8 * NF]
            ).then_inc(ds_ring, 16)
            fin("m5h1", 9)   # og5
            gpsimd.wait_ge(od_sem, 6)
            gpsimd.dma_start(
                out_r[5][:], ot_sb[:, 10 * NF : 12 * NF]
            ).then_inc(ds_ring, 16)
            fin("m6h1", 10)  # og6
            # m7 h0 second half (cols 256:512)
            gpsimd.wait_ge(od_sem, 9)
            gpsimd.dma_start(
                out_r[7][:, HNF:NF],
                ot_sb[:, 14 * NF + HNF : 15 * NF],
            ).then_inc(ds_ring, 16)
            gpsimd.wait_ge(ds_ring, 4 * 16)

        @block.tensor
        def _(tensor):
            # warmups: keep PE busy (and the HAM ramp alive) from
            # preamble end until the first chunks land (~10us).
            for _ in range(N_WU):
                nc.tensor.matmul(
                    ps[0][:, 0:256],
                    wu_sb[:, :, 0:P],
                    wu_sb[:, :, :],
                    start=True,
                    stop=True,
                    perf_mode=_DR,
                )
            # pass 1: m0-3, d outer / h outer / m inner.
            # banks: (m,h0)->m, (m,h1)->4+m
            for d in range(DP_TILES):
                tensor.wait_ge(xta_sems[d], 16)
                tensor.wait_ge(cta_sems[d], 16)
                for m in range(4):
                    mm = nc.tensor.matmul(
                        ps[m][:],
                        ct_sb[d][:, :, m * P : (m + 1) * P],
                        xt_sb[d][:, :, 0:NF],
                        start=(d == 0),
                        stop=(d == DP_TILES - 1),
                        perf_mode=_DR,
                    )
                    if d == DP_TILES - 1:
                        mm.then_inc(mm_sem, 1)
                tensor.wait_ge(xtb_sems[d], 16)
                for m in range(4):
                    mm = nc.tensor.matmul(
                        ps[4 + m][:],
                        ct_sb[d][:, :, m * P : (m + 1) * P],
                        xt_sb[d][:, :, NF:NSH],
                        start=(d == 0),
                        stop=(d == DP_TILES - 1),
                        perf_mode=_DR,
                    )
                    if d == DP_TILES - 1:
                        mm.then_inc(mm_sem, 1)
            # pass 2: m4-7, m outer / d inner; h0+h1 interleaved per d.
            # bank handoff: m4->(0,1) after D(m0h0)/A(m1h0);
            # m5->(2,3) after A(m2h0),A(m3h0); m6->(4,5); m7 h0->6,
            # h1a->7 (ex m3h1), h1b->0 (ex m4 h0, od>=5).
            tensor.wait_ge(ctb_sem, 64)
            for mi, m in enumerate((4, 5, 6)):
                if mi == 0:
                    tensor.wait_ge(od_sem, 1)
                    tensor.wait_ge(ob_sem, 1)
                elif mi == 1:
                    tensor.wait_ge(ob_sem, 3)
                else:
                    tensor.wait_ge(ob_sem, 5)
                b0, b1 = 2 * mi, 2 * mi + 1
                for d in range(DP_TILES):
                    mm0 = nc.tensor.matmul(
                        ps[b0][:],
                        ct_sb[d][:, :, m * P : (m + 1) * P],
                        xt_sb[d][:, :, 0:NF],
                        start=(d == 0),
                        stop=(d == DP_TILES - 1),
                        perf_mode=_DR,
                    )
                    mm1 = nc.tensor.matmul(
                        ps[b1][:],
                        ct_sb[d][:, :, m * P : (m + 1) * P],
                        xt_sb[d][:, :, NF:NSH],
                        start=(d == 0),
                        stop=(d == DP_TILES - 1),
                        perf_mode=_DR,
                    )
                    if d == DP_TILES - 1:
                        mm0.then_inc(mm_sem, 1)
                        mm1.then_inc(mm_sem, 1)
            # m7: h0 whole (bank6), then h1 as two 256-col sub-groups
            tensor.wait_ge(ob_sem, 6)  # bank6 (ex m2h1) free
            for d in range(DP_TILES):
                mm = nc.tensor.matmul(
                    ps[6][:],
                    ct_sb[d][:, :, 7 * P : 8 * P],
                    xt_sb[d][:, :, 0:NF],
                    start=(d == 0),
                    stop=(d == DP_TILES - 1),
                    perf_mode=_DR,
                )
            mm.then_inc(mm_sem, 1)  # 15
            tensor.wait_ge(ob_sem, 7)  # bank7 (ex m3h1) free
            for d in range(DP_TILES):
                mm = nc.tensor.matmul(
                    ps[7][:, 0:HNF],
                    ct_sb[d][:, :, 7 * P : 8 * P],
                    xt_sb[d][:, :, NF : NF + HNF],
                    start=(d == 0),
                    stop=(d == DP_TILES - 1),
                    perf_mode=_DR,
                )
            mm.then_inc(mm_sem, 1)  # 16
            tensor.wait_ge(od_sem, 5)  # bank0 (ex m4h0) free
            for d in range(DP_TILES):
                mm = nc.tensor.matmul(
                    ps[0][:, 0:HNF],
                    ct_sb[d][:, :, 7 * P : 8 * P],
                    xt_sb[d][:, :, NF + HNF : NSH],
                    start=(d == 0),
                    stop=(d == DP_TILES - 1),
                    perf_mode=_DR,
                )
            mm.then_inc(mm_sem, 1)  # 17

        @block.vector
        def _(vector):
            vector.wait_ge(bc_sem, 16)
            vector.wait_ge(const_sem, 16)

            def direct(name, mmw, bank, lo, w):
                # ot = (ps + ncsq_m) + nxsq, straight from PSUM
                vector.wait_ge(mm_sem, mmw)
                oc, _ = _oc(name)
                m = int(name[1])
                nc.vector.scalar_tensor_tensor(
                    ot_sb[:, oc : oc + w],
                    ps[bank][:, lo : lo + w],
                    ncsq_sb[:, m : m + 1],
                    _nxsq_slice(name),
                    op0=mybir.AluOpType.add,
                    op1=mybir.AluOpType.add,
                ).then_inc(od_sem, 1)

            def fin(name, ob_cnt):
                vector.wait_ge(ob_sem, ob_cnt)
                lo, w = _oc(name)
                nc.vector.tensor_add(
                    ot_sb[:, lo : lo + w],
                    tmp_sb[:, BUF[name], 0:w],
                    _nxsq_slice(name),
                ).then_inc(od_sem, 1)

            direct("m0h0", 1, 0, 0, NF)   # od1
            fin("m1h0", 1)                # od2
            fin("m2h0", 2)                # od3
            fin("m3h0", 3)                # od4
            direct("m4h0", 9, 0, 0, NF)   # od5
            direct("m5h0", 11, 2, 0, NF)  # od6
            fin("m4h1", 8)                # od7
            direct("m6h0", 13, 4, 0, NF)  # od8
            direct("m7h0", 15, 6, 0, NF)  # od9
            direct("m7h1b", 17, 0, 0, HNF)  # od10
            fin("m7h1a", 11)              # od11

    nc.compile()
    return nc


def _get_nc():
    if not hasattr(_cache, "nc"):
        _cache.nc = _build_nc()
    return _cache.nc


def _to_dr_layout(a_t):
    """[D, F] -> [DP_TILES, P, 2, F]: d = dp*256 + i*128 + p."""
    F = a_t.shape[1]
    return np.ascontiguousarray(
        a_t.reshape(DP_TILES, 2, P, F).transpose(0, 2, 1, 3)
    )


def kernel(inputs, centers, _trace=False):
    inputs = np.asarray(inputs, dtype=np.float32)
    centers = np.asarray(centers, dtype=np.float32)

    csq = np.sum(centers.astype(np.float64) ** 2, axis=1)
    xsq = np.sum(inputs.astype(np.float64) ** 2, axis=1)

    ct8 = _to_dr_layout(centers.T.astype(_NP_DT))
    cta = np.ascontiguousarray(ct8[:, :, :, 0:NF])
    ctb = np.ascontiguousarray(ct8[:, :, :, NF:K])
    xt8_full = (2.0 * inputs).T.astype(_NP_DT)  # [D, N]
    ncsq = np.ascontiguousarray((-csq).reshape(M_TILES, P).T.astype(np.float32))

    in_maps = []
    for i in range(N_CORES):
        sl = slice(i * NSH, (i + 1) * NSH)
        xt8 = _to_dr_layout(xt8_full[:, sl])
        in_maps.append(
            {
                "cta": cta,
                "ctb": ctb,
                "xta": np.ascontiguousarray(xt8[:, :, :, 0:NF]),
                "xtb": np.ascontiguousarray(xt8[:, :, :, NF:NSH]),
                "ncsq": ncsq,
                "nxsq": np.ascontiguousarray(
                    np.broadcast_to((-xsq[sl]).astype(np.float16), (P, NSH))
                ),
            }
        )

    nc = _get_nc()
    try:
        res = run_bass_kernel_spmd(
            nc, in_maps, core_ids=list(range(N_CORES)), trace=_trace
        )
    except ModuleNotFoundError:
        res = run_bass_kernel_spmd(
            nc, in_maps, core_ids=list(range(N_CORES)), trace=False
        )
    if _trace:
        kernel.last_results = res
    return np.concatenate(
        [np.asarray(r["out"]).astype(np.float32) for r in res.results], axis=1
    )


# revision 12
# speedup vs baseline: 1.2553x; 1.2553x over previous
"""Trainium2 Bass kernel for nn_CentersDistance (retrieval_knn).

logits[k, n] = -||centers[k] - inputs[n]||^2
             = 2*(centers @ inputs.T)[k, n] - ||centers[k]||^2 - ||inputs[n]||^2

Strategy (8 NeuronCores, data-parallel over the N=8192 inputs): fp8
DoubleRow GEMM (64 matmuls of [128p x 256c x 512f], PSUM fp32), exact
f64 norm terms added in a PSUM->SBUF epilogue, bf16/fp16 output.

v2 changes vs the 31.7us baseline (trace-driven):
  * loads are 128KB chunks with per-chunk semaphores, ordered so the
    d0 tile pair (xta[0] on the Sync HW queue, cta[0] on the Scalar HW
    queue) is the FIRST thing each queue moves.  Measured: per-queue
    streaming is only ~95-130 B/ns and first packets appear ~1.4us
    after dispatch, so the baseline's 256KB-tile + ncsq-first order
    made the first real matmul wait until 12.1us; chunking pulls it
    to ~10.0us.
  * ct is split m0-3 (cta, pass 1) / m4-7 (ctb, pass 2): pass 1 only
    needs half of each ct d-tile, pass 2's half arrives later with
    slack (Scalar: ctb0,1; ring: ctb2,3).
  * pass 1 is d-outer / h-outer / m-inner (first 4 matmuls need only
    xta[0]+cta[0] = 256KB); pass 2 is m-outer / d-inner so each
    m-tile's two groups retire every ~1.7us for store overlap, with
    PSUM bank handoff waits matched to the drain schedule.
  * the LAST m-tile (m7) runs h0 (bank6), then h1 as two 256-col
    sub-groups in two different freed banks (7 and 0), so the final
    drain after the last matmul is a [128,256] op, and the final
    stores are 64KB quarters spread across all three DMA queues.
  * epilogue is spread over three engines: DVE direct STTs + finishes,
    Act PSUM drains (Identity + ncsq bias -> fp16 tmp), GpSimd
    tensor_add finishes (SBUF-only).  Assignments are chosen so no
    engine queue backs up at the tail and Act/DVE never touch the
    same PSUM bank concurrently (P10: concurrent access of one bank
    from two engines is unsafe; PE-write + engine-read is fatal).
  * PE warmup matmuls (N=256, ~213ns each cold) bridge the preamble
    until the first chunks land and keep the HAM clock ramp running.

Measured window note: exec_time = last semaphore teardown event minus
first post-preamble instruction; the ~6.8us all-sems reset storm the
NEFF epilogue emits is fixed-length, so kernel-body savings move the
reported number 1:1.

Accuracy: identical quantization to baseline (fp8e4m3 cross term,
exact f64 norms, fp16 store) -> absmax/scale ~5.2e-3 vs 2e-2 gate.
"""

import threading
from contextlib import ExitStack

import numpy as np
import ml_dtypes

import concourse.mybir as mybir
from concourse import bacc
from concourse.bass_utils import run_bass_kernel_spmd

N_CORES = 8
N, K, D = 8192, 1024, 1024
NSH = N // N_CORES  # per-core slab of inputs
P = 128             # SBUF partitions
NF = 512            # matmul moving free dim (one fp32 PSUM bank)
HNF = NF // 2

DP_TILES = D // (2 * P)  # 4 double-row contraction tiles (256 deep each)
M_TILES = K // P         # 8 center tiles

N_WU = 14           # warmup matmuls (N=256, ~213ns each cold)
N_TMP = 6           # fp16 staging buffers for Act->finish pipeline

_DT = mybir.dt.float8e4
_NP_DT = ml_dtypes.float8_e4m3
_OUT_DT = mybir.dt.float16
_DR = mybir.MatmulPerfMode.DoubleRow

_cache = threading.local()


def _build_nc():
    nc = bacc.Bacc(
        "TRN2", target_bir_lowering=False, debug=False, num_devices=N_CORES
    )
    cta = nc.dram_tensor("cta", [DP_TILES, P, 2, NF], _DT, kind="ExternalInput").ap()
    ctb = nc.dram_tensor("ctb", [DP_TILES, P, 2, NF], _DT, kind="ExternalInput").ap()
    xta = nc.dram_tensor("xta", [DP_TILES, P, 2, NF], _DT, kind="ExternalInput").ap()
    xtb = nc.dram_tensor("xtb", [DP_TILES, P, 2, NF], _DT, kind="ExternalInput").ap()
    ncsq = nc.dram_tensor(
        "ncsq", [P, M_TILES], mybir.dt.float32, kind="ExternalInput"
    ).ap()
    nxsq = nc.dram_tensor(
        "nxsq", [P, NSH], mybir.dt.float16, kind="ExternalInput"
    ).ap()
    out = nc.dram_tensor("out", [K, NSH], _OUT_DT, kind="ExternalOutput").ap()
    out_r = out.rearrange("(m p) n -> m p n", p=P)

    with (
        nc.sbuf_tensor("wu_sb", [P, 2, 256], _DT) as wu_sb,
        nc.sbuf_tensor("ncsq_sb", [P, M_TILES], mybir.dt.float32) as ncsq_sb,
        nc.sbuf_tensor("nxsq_sb", [P, NSH], mybir.dt.float16) as nxsq_sb,
        nc.sbuf_tensor("tmp_sb", [P, N_TMP, NF], mybir.dt.float16) as tmp_sb,
        # ot layout: [(m, h)] -> col block 2m+h, so each m-tile's store
        # is one contiguous [128, 1024] fp16 DMA (2KB lines).
        nc.sbuf_tensor("ot_sb", [P, 2 * M_TILES * NF], _OUT_DT) as ot_sb,
        ExitStack() as stack,
        nc.semaphore("const_sem") as const_sem,  # ncsq landed
        nc.semaphore("bc_sem") as bc_sem,        # nxsq landed
        nc.semaphore("mm_sem") as mm_sem,        # group stop-matmul retired
        nc.semaphore("ob_sem") as ob_sem,        # Act drain done (PSUM free)
        nc.semaphore("od_sem") as od_sem,        # DVE finalize count
        nc.semaphore("og_sem") as og_sem,        # GpSimd finalize count
        nc.semaphore("ctb_sem") as ctb_sem,      # all 4 chunks -> 64
        nc.semaphore("ds_sync") as ds_sync,
        nc.semaphore("ds_ring") as ds_ring,
        nc.semaphore("ds_scalar") as ds_scalar,
        nc.Block() as block,
    ):
        cta_sems = [
            stack.enter_context(nc.semaphore(f"cta_sem{d}")) for d in range(DP_TILES)
        ]
        xta_sems = [
            stack.enter_context(nc.semaphore(f"xta_sem{d}")) for d in range(DP_TILES)
        ]
        xtb_sems = [
            stack.enter_context(nc.semaphore(f"xtb_sem{d}")) for d in range(DP_TILES)
        ]
        ct_sb = [
            stack.enter_context(nc.sbuf_tensor(f"ct_sb{d}", [P, 2, K], _DT))
            for d in range(DP_TILES)
        ]
        xt_sb = [
            stack.enter_context(nc.sbuf_tensor(f"xt_sb{d}", [P, 2, NSH], _DT))
            for d in range(DP_TILES)
        ]
        ps = [
            stack.enter_context(nc.psum_tensor(f"ps{b}", [P, NF], mybir.dt.float32))
            for b in range(8)
        ]

        # ---- schedules -------------------------------------------------
        # mm_sem increment order (one inc per group's stop matmul):
        #  1 m0h0  2 m1h0  3 m2h0  4 m3h0  5 m0h1  6 m1h1  7 m2h1  8 m3h1
        #  9 m4h0 10 m4h1 11 m5h0 12 m5h1 13 m6h0 14 m6h1
        # 15 m7h0 16 m7h1a(bank7, cols 512:768) 17 m7h1b(bank0, 768:1024)
        #
        # Act drain order (ob):  m1h0, m2h0, m3h0, m0h1, m1h1, m2h1,
        #   m3h1, m4h1, m5h1, m6h1, m7h1a           (11 drains -> tmp)
        # DVE order (od): D m0h0, F m1h0, F m2h0, F m3h0, D m4h0,
        #   D m5h0, F m4h1, D m6h0, D m7h0, D m7h1b, F m7h1a
        # GpSimd finishes (og): m0h1, m1h1, m2h1, m3h1, m5h1, m6h1

        ACT_LIST = [  # (name, mm_wait, bank, psum_lo, psum_hi, ob_after)
            ("m1h0", 2, 1, 0, NF, 1),
            ("m2h0", 3, 2, 0, NF, 2),
            ("m3h0", 4, 3, 0, NF, 3),
            ("m2h1", 7, 6, 0, NF, 4),
            ("m3h1", 8, 7, 0, NF, 5),
            ("m0h1", 5, 4, 0, NF, 6),
            ("m1h1", 6, 5, 0, NF, 7),
            ("m4h1", 10, 1, 0, NF, 8),
            ("m5h1", 12, 3, 0, NF, 9),
            ("m6h1", 14, 5, 0, NF, 10),
            ("m7h1a", 16, 7, 0, HNF, 11),
        ]
        # ncsq column for each drained group
        ACT_M = {
            "m1h0": 1, "m2h0": 2, "m3h0": 3, "m0h1": 0, "m1h1": 1,
            "m2h1": 2, "m3h1": 3, "m4h1": 4, "m5h1": 5, "m6h1": 6,
            "m7h1a": 7,
        }
        # output column block (in units of HNF) and width for each group
        def _oc(name):
            m = int(name[1])
            if name == "m7h1a":
                return (2 * m + 1) * NF, HNF
            if name == "m7h1b":
                return (2 * m + 1) * NF + HNF, HNF
            h = int(name[3])
            return (2 * m + h) * NF, NF

        # tmp slot per Act drain + reuse wait: slot j%6; for j>=6 wait
        # until the finish consuming ACT_LIST[j-6]'s tmp has run.
        # GpSimd TENSOR_TENSOR measured ~3.6x slower than DVE (1.5us per
        # [128,512]) -> it gets only the two highest-slack finishes.
        FIN = {
            "m1h0": ("od", 2), "m2h0": ("od", 3), "m3h0": ("od", 4),
            "m2h1": ("od", 5), "m3h1": ("od", 6), "m0h1": ("og", 1),
            "m1h1": ("og", 2), "m4h1": ("od", 9), "m5h1": ("od", 11),
            "m6h1": ("od", 12), "m7h1a": ("od", 15),
        }
        BUF = {it[0]: j % N_TMP for j, it in enumerate(ACT_LIST)}

        def _nxsq_slice(name):
            lo, w = _oc(name)
            # output block (2m+h)*NF+off maps to slab cols h*NF+off
            m = int(name[1])
            slab = lo - 2 * m * NF
            return nxsq_sb[:, slab : slab + w]

        # ---- engine programs -------------------------------------------

        @block.sync
        def _(sync):
            # critical chunk first: xta[0] feeds the first 4 matmuls
            for d in range(DP_TILES):
                sync.dma_start(
                    xt_sb[d][:, :, 0:NF], xta[d]
                ).then_inc(xta_sems[d], 16)
            sync.dma_start(ncsq_sb[:], ncsq).then_inc(const_sem, 16)
            # stores: m0, m2, m4, m6, then m7h1a
            sync.wait_ge(od_sem, 1)
            sync.wait_ge(og_sem, 1)
            sync.dma_start(out_r[0][:], ot_sb[:, 0 : 2 * NF]).then_inc(ds_sync, 16)
            sync.wait_ge(od_sem, 5)
            sync.dma_start(
                out_r[2][:], ot_sb[:, 4 * NF : 6 * NF]
            ).then_inc(ds_sync, 16)
            sync.wait_ge(od_sem, 9)
            sync.dma_start(
                out_r[4][:], ot_sb[:, 8 * NF : 10 * NF]
            ).then_inc(ds_sync, 16)
            sync.wait_ge(od_sem, 12)
            sync.dma_start(
                out_r[6][:], ot_sb[:, 12 * NF : 14 * NF]
            ).then_inc(ds_sync, 16)
            # m7 h1a (cols 512:768)
            sync.wait_ge(od_sem, 15)
            sync.dma_start(
                out_r[7][:, NF : NF + HNF],
                ot_sb[:, 15 * NF : 15 * NF + HNF],
            ).then_inc(ds_sync, 16)
            sync.wait_ge(ds_sync, 5 * 16)

        @block.scalar
        def _(scalar):
            for d in range(DP_TILES):
                scalar.dma_start(
                    ct_sb[d][:, :, 0:NF], cta[d]
                ).then_inc(cta_sems[d], 16)
            for d in (0, 1):
                scalar.dma_start(
                    ct_sb[d][:, :, NF:K], ctb[d]
                ).then_inc(ctb_sem, 16)
            # Act drains: tmp = Identity(ps + ncsq_bias), fp32 PSUM -> fp16
            scalar.wait_ge(const_sem, 16)
            for j, (name, mmw, bank, lo, hi, _ob) in enumerate(ACT_LIST):
                if j >= N_TMP:
                    dep, cnt = FIN[ACT_LIST[j - N_TMP][0]]
                    scalar.wait_ge(od_sem if dep == "od" else og_sem, cnt)
                scalar.wait_ge(mm_sem, mmw)
                nc.scalar.add(
                    tmp_sb[:, BUF[name], 0 : hi - lo],
                    ps[bank][:, lo:hi],
                    ncsq_sb[:, ACT_M[name] : ACT_M[name] + 1],
                ).then_inc(ob_sem, 1)
            # store m7h0a (cols 0:256) and m7h1b (cols 768:1024)
            scalar.wait_ge(od_sem, 13)
            scalar.dma_start(
                out_r[7][:, 0:HNF],
                ot_sb[:, 14 * NF : 14 * NF + HNF],
            ).then_inc(ds_scalar, 16)
            scalar.wait_ge(od_sem, 14)
            scalar.dma_start(
                out_r[7][:, NF + HNF : 2 * NF],
                ot_sb[:, 15 * NF + HNF : 16 * NF],
            ).then_inc(ds_scalar, 16)
            scalar.wait_ge(ds_scalar, 2 * 16)

        @block.gpsimd
        def _(gpsimd):
            for d in range(DP_TILES):
                gpsimd.dma_start(
                    xt_sb[d][:, :, NF:NSH], xtb[d]
                ).then_inc(xtb_sems[d], 16)
            for d in (2, 3):
                gpsimd.dma_start(
                    ct_sb[d][:, :, NF:K], ctb[d]
                ).then_inc(ctb_sem, 16)
            gpsimd.dma_start(nxsq_sb[:], nxsq).then_inc(bc_sem, 16)

            def fin(name, ob_cnt):
                gpsimd.wait_ge(ob_sem, ob_cnt)
                lo, w = _oc(name)
                nc.gpsimd.tensor_add(
                    ot_sb[:, lo : lo + w],
                    tmp_sb[:, BUF[name], 0:w],
                    _nxsq_slice(name),
                ).then_inc(og_sem, 1)

            gpsimd.wait_ge(bc_sem, 16)
            fin("m0h1", 6)   # og1
            fin("m1h1", 7)   # og2
            gpsimd.wait_ge(od_sem, 2)
            gpsimd.dma_start(
                out_r[1][:], ot_sb[:, 2 * NF : 4 * NF]
            ).then_inc(ds_ring, 16)
            gpsimd.wait_ge(od_sem, 6)
            gpsimd.dma_start(
                out_r[3][:], ot_sb[:, 6 * NF : 8 * NF]
            ).then_inc(ds_ring, 16)
            gpsimd.wait_ge(od_sem, 11)
            gpsimd.dma_start(
                out_r[5][:], ot_sb[:, 10 * NF : 12 * NF]
            ).then_inc(ds_ring, 16)
            # m7 h0 second half (cols 256:512)
            gpsimd.wait_ge(od_sem, 13)
            gpsimd.dma_start(
                out_r[7][:, HNF:NF],
                ot_sb[:, 14 * NF + HNF : 15 * NF],
            ).then_inc(ds_ring, 16)
            gpsimd.wait_ge(ds_ring, 4 * 16)

        @block.tensor
        def _(tensor):
            # warmups: keep PE busy (and the HAM ramp alive) from
            # preamble end until the first chunks land (~10us).
            for _ in range(N_WU):
                nc.tensor.matmul(
                    ps[0][:, 0:256],
                    wu_sb[:, :, 0:P],
                    wu_sb[:, :, :],
                    start=True,
                    stop=True,
                    perf_mode=_DR,
                )
            # pass 1: m0-3, d outer / h outer / m inner.
            # banks: (m,h0)->m, (m,h1)->4+m
            for d in range(DP_TILES):
                tensor.wait_ge(xta_sems[d], 16)
                tensor.wait_ge(cta_sems[d], 16)
                for m in range(4):
                    mm = nc.tensor.matmul(
                        ps[m][:],
                        ct_sb[d][:, :, m * P : (m + 1) * P],
                        xt_sb[d][:, :, 0:NF],
                        start=(d == 0),
                        stop=(d == DP_TILES - 1),
                        perf_mode=_DR,
                    )
                    if d == DP_TILES - 1:
                        mm.then_inc(mm_sem, 1)
                tensor.wait_ge(xtb_sems[d], 16)
                for m in range(4):
                    mm = nc.tensor.matmul(
                        ps[4 + m][:],
                        ct_sb[d][:, :, m * P : (m + 1) * P],
                        xt_sb[d][:, :, NF:NSH],
                        start=(d == 0),
                        stop=(d == DP_TILES - 1),
                        perf_mode=_DR,
                    )
                    if d == DP_TILES - 1:
                        mm.then_inc(mm_sem, 1)
            # pass 2: m4-7, m outer / d inner; h0+h1 interleaved per d.
            # bank handoff: m4->(0,1) after D(m0h0)/A(m1h0);
            # m5->(2,3) after A(m2h0),A(m3h0); m6->(4,5); m7 h0->6,
            # h1a->7 (ex m3h1), h1b->0 (ex m4 h0, od>=5).
            tensor.wait_ge(ctb_sem, 64)
            for mi, m in enumerate((4, 5, 6)):
                if mi == 0:
                    tensor.wait_ge(od_sem, 1)
                    tensor.wait_ge(ob_sem, 1)
                elif mi == 1:
                    tensor.wait_ge(ob_sem, 3)
                else:
                    tensor.wait_ge(ob_sem, 7)
                b0, b1 = 2 * mi, 2 * mi + 1
                for d in range(DP_TILES):
                    mm0 = nc.tensor.matmul(
                        ps[b0][:],
                        ct_sb[d][:, :, m * P : (m + 1) * P],
                        xt_sb[d][:, :, 0:NF],
                        start=(d == 0),
                        stop=(d == DP_TILES - 1),
                        perf_mode=_DR,
                    )
                    mm1 = nc.tensor.matmul(
                        ps[b1][:],
                        ct_sb[d][:, :, m * P : (m + 1) * P],
                        xt_sb[d][:, :, NF:NSH],
                        start=(d == 0),
                        stop=(d == DP_TILES - 1),
                        perf_mode=_DR,
                    )
                    if d == DP_TILES - 1:
                        mm0.then_inc(mm_sem, 1)
                        mm1.then_inc(mm_sem, 1)
            # m7: h0 whole (bank6), then h1 as two 256-col sub-groups
            tensor.wait_ge(ob_sem, 4)  # bank6 (ex m2h1) free
            for d in range(DP_TILES):
                mm = nc.tensor.matmul(
                    ps[6][:],
                    ct_sb[d][:, :, 7 * P : 8 * P],
                    xt_sb[d][:, :, 0:NF],
                    start=(d == 0),
                    stop=(d == DP_TILES - 1),
                    perf_mode=_DR,
                )
            mm.then_inc(mm_sem, 1)  # 15
            tensor.wait_ge(ob_sem, 5)  # bank7 (ex m3h1) free
            for d in range(DP_TILES):
                mm = nc.tensor.matmul(
                    ps[7][:, 0:HNF],
                    ct_sb[d][:, :, 7 * P : 8 * P],
                    xt_sb[d][:, :, NF : NF + HNF],
                    start=(d == 0),
                    stop=(d == DP_TILES - 1),
                    perf_mode=_DR,
                )
            mm.then_inc(mm_sem, 1)  # 16
            tensor.wait_ge(od_sem, 7)  # bank0 (ex m4h0) free
            for d in range(DP_TILES):
                mm = nc.tensor.matmul(
                    ps[0][:, 0:HNF],
                    ct_sb[d][:, :, 7 * P : 8 * P],
                    xt_sb[d][:, :, NF + HNF : NSH],
                    start=(d == 0),
                    stop=(d == DP_TILES - 1),
                    perf_mode=_DR,
                )
            mm.then_inc(mm_sem, 1)  # 17

        @block.vector
        def _(vector):
            vector.wait_ge(bc_sem, 16)
            vector.wait_ge(const_sem, 16)

            def direct(name, mmw, bank, lo, w):
                # ot = (ps + ncsq_m) + nxsq, straight from PSUM
                vector.wait_ge(mm_sem, mmw)
                oc, _ = _oc(name)
                m = int(name[1])
                nc.vector.scalar_tensor_tensor(
                    ot_sb[:, oc : oc + w],
                    ps[bank][:, lo : lo + w],
                    ncsq_sb[:, m : m + 1],
                    _nxsq_slice(name),
                    op0=mybir.AluOpType.add,
                    op1=mybir.AluOpType.add,
                ).then_inc(od_sem, 1)

            def fin(name, ob_cnt):
                vector.wait_ge(ob_sem, ob_cnt)
                lo, w = _oc(name)
                nc.vector.tensor_add(
                    ot_sb[:, lo : lo + w],
                    tmp_sb[:, BUF[name], 0:w],
                    _nxsq_slice(name),
                ).then_inc(od_sem, 1)

            direct("m0h0", 1, 0, 0, NF)   # od1
            fin("m1h0", 1)                # od2
            fin("m2h0", 2)                # od3
            fin("m3h0", 3)                # od4
            fin("m2h1", 4)                # od5
            fin("m3h1", 5)                # od6
            direct("m4h0", 9, 0, 0, NF)   # od7
            direct("m5h0", 11, 2, 0, NF)  # od8
            fin("m4h1", 8)                # od9
            direct("m6h0", 13, 4, 0, NF)  # od10
            fin("m5h1", 9)                # od11
            fin("m6h1", 10)               # od12
            direct("m7h0", 15, 6, 0, NF)  # od13
            direct("m7h1b", 17, 0, 0, HNF)  # od14
            fin("m7h1a", 11)              # od15

    nc.compile()
    return nc


def _get_nc():
    if not hasattr(_cache, "nc"):
        _cache.nc = _build_nc()
    return _cache.nc


def _to_dr_layout(a_t):
    """[D, F] -> [DP_TILES, P, 2, F]: d = dp*256 + i*128 + p."""
    F = a_t.shape[1]
    return np.ascontiguousarray(
        a_t.reshape(DP_TILES, 2, P, F).transpose(0, 2, 1, 3)
    )


def kernel(inputs, centers, _trace=False):
    inputs = np.asarray(inputs, dtype=np.float32)
    centers = np.asarray(centers, dtype=np.float32)

    csq = np.sum(centers.astype(np.float64) ** 2, axis=1)
    xsq = np.sum(inputs.astype(np.float64) ** 2, axis=1)

    ct8 = _to_dr_layout(centers.T.astype(_NP_DT))
    cta = np.ascontiguousarray(ct8[:, :, :, 0:NF])
    ctb = np.ascontiguousarray(ct8[:, :, :, NF:K])
    xt8_full = (2.0 * inputs).T.astype(_NP_DT)  # [D, N]
    ncsq = np.ascontiguousarray((-csq).reshape(M_TILES, P).T.astype(np.float32))

    in_maps = []
    for i in range(N_CORES):
        sl = slice(i * NSH, (i + 1) * NSH)
        xt8 = _to_dr_layout(xt8_full[:, sl])
        in_maps.append(
            {
                "cta": cta,
                "ctb": ctb,
                "xta": np.ascontiguousarray(xt8[:, :, :, 0:NF]),
                "xtb": np.ascontiguousarray(xt8[:, :, :, NF:NSH]),
                "ncsq": ncsq,
                "nxsq": np.ascontiguousarray(
                    np.broadcast_to((-xsq[sl]).astype(np.float16), (P, NSH))
                ),
            }
        )

    nc = _get_nc()
    try:
        res = run_bass_kernel_spmd(
            nc, in_maps, core_ids=list(range(N_CORES)), trace=_trace
        )
    except ModuleNotFoundError:
        res = run_bass_kernel_spmd(
            nc, in_maps, core_ids=list(range(N_CORES)), trace=False
        )
    if _trace:
        kernel.last_results = res
    return np.concatenate(
        [np.asarray(r["out"]).astype(np.float32) for r in res.results], axis=1
    )
